# revision 34
# baseline (speedup 1.0000x reference)
"""Trainium2 Bass kernel for nn_Attention_28269474742408.

Single-layer attention block: qkv projections -> softmax attention ->
layernorm -> output projection, for x [8, 1024, 768] (B=8, N=1024, C=768,
H=12 heads, D=64).

Strategy: data parallel over the batch — one batch element per NeuronCore
(8 cores). Everything on-chip per core; no collectives.

Per-core structure (all channel-major, "T" = [channel, token]):
  - Host pre-transposes x[b] -> xT [768, 1024] and all weights -> W.T so
    projections/attention never need on-device transposes.
  - v is projected token-major [1024, 780]: 12 heads x (64 v-cols + a ones
    column); the ones column makes the PV matmul emit softmax denominators
    for free.
  - The attention is software-pipelined one head ahead: while head H's PV
    matmuls accumulate, head H+1's scores matmuls + exp run, so the
    ScalarE exp stream (the per-head rate limiter) stays saturated.  The
    v projection and head 0's scores/exp are interleaved in the prologue
    (scores on their own PSUM ring, v-chunks/projections on a
    prologue-scoped ring), and the next pair's q/k projection groups are
    slotted into the PV stream.
  - Scores are computed as S.T [m, n] per head; softmax skips
    max-subtraction (scores bounded ~|3|, exp can't overflow) so exp needs
    no cross-partition reduction.
  - attnT [64+1, 1024] per head accumulates over m-tiles in PSUM
    (flash-style); the division by denominators uses a fast approx
    reciprocal (SBUF-side: the custom-DVE op misbehaves on hw reading
    PSUM) and a DRAM-bounced row broadcast.
  - LayerNorm is folded into the output projection: gamma/beta folded into
    Wo/bo on the host; mean/var via ones-matmuls (cross-partition sums);
    the -mean*colsum(Wo) rank-1 correction rides the output matmul as an
    extra K=1 accumulation; the per-token rsqrt scale and the +bo_eff bias
    are both applied on eviction by one scalar_tensor_tensor
    ((po * rstd) + bo), with rstd computed in [128, NT] shape (DMA-bounce
    transpose, then sqrt + fast reciprocal on all 128 lanes).
  - Weight loads ride the otherwise-idle GpSimd DMA queue; xT tiles
    alternate between the sync and scalar DMA queues.
"""
import os

import numpy as np

import concourse.bacc as bacc
import concourse.bass as bass
import concourse.tile as tile
from concourse import bass_isa
from concourse import mybir
from concourse.bass_utils import run_bass_kernel_spmd

F32 = mybir.dt.float32
F32R = mybir.dt.float32r
F16 = mybir.dt.float16
AF = mybir.ActivationFunctionType
OP = mybir.AluOpType

K_GPSIMD_DMA = os.environ.get("K_GPSIMD_DMA", "1") == "1"

B, N, C, H, D = 8, 1024, 768, 12, 64
KT = C // 128          # 6 channel tiles
NT = N // 128          # 8 token tiles
NP = H // 2            # 6 head pairs
VW = H * (D + 1)       # 780: v plus per-head ones column
SCALE = D ** -0.5
EPS = 1e-5


def build_kernel():
    nc = bacc.Bacc("TRN2", target_bir_lowering=False)
    wdma = (lambda **kw: nc.gpsimd.dma_start(**kw)) if K_GPSIMD_DMA else \
        (lambda **kw: nc.sync.dma_start(**kw))

    xt_d = nc.dram_tensor("xt", (C, N), F16, kind="ExternalInput")
    wv_d = nc.dram_tensor("wv", (C, VW), F16, kind="ExternalInput")
    wqk_d = nc.dram_tensor("wqk", (C, 2 * C), F16, kind="ExternalInput")
    wo_d = nc.dram_tensor("wo", (C, C), F16, kind="ExternalInput")
    extra_d = nc.dram_tensor("extra", (2, C), F16, kind="ExternalInput")
    bqs_d = nc.dram_tensor("bqs", (C,), F32, kind="ExternalInput")
    bve_d = nc.dram_tensor("bve", (VW,), F16, kind="ExternalInput")
    ones_d = nc.dram_tensor("onesd", (128,), F16, kind="ExternalInput")

    y_d = nc.dram_tensor("y", (N, C), F32, kind="ExternalOutput")
    rscr = nc.dram_tensor("rscr", (H, N), F32)     # internal: recip bounce
    ascr = nc.dram_tensor("ascr", (N,), F32)       # internal: var bounce

    with tile.TileContext(nc) as tc:
        with tc.tile_pool(name="persist", bufs=1) as pp, \
             tc.tile_pool(name="attp", bufs=1) as attp, \
             tc.tile_pool(name="wop", bufs=1) as wop, \
             tc.tile_pool(name="sqp", bufs=1) as sqp:

            # ---- constants (loads emitted after the xT DMA, below) ----
            extra_t = pp.tile([2, C], F16, tag="extra", name="extra")
            bqs_t = [pp.tile([128, 1], F32, tag=f"bqs{m}", name=f"bqs{m}")
                     for m in range(NP)]
            ones_col = pp.tile([128, 1], F16, tag="ones_col", name="ones_col")
            eps_row = pp.tile([1, 1], F32, tag="eps_row", name="eps_row")
            nc.vector.memset(eps_row, EPS)
            # broadcast biases (applied on eviction; DMAs emitted later so
            # they don't delay the xT load)
            bve_bc = pp.tile([128, VW], F16, tag="bve_bc", name="bve_bc")
            bo_bc = pp.tile([128, C], F16, tag="bo_bc", name="bo_bc")

            att = [attp.tile([128, N], F16, tag=f"att{k}", name=f"att{k}")
                   for k in range(KT)]
            f_t = pp.tile([1, N], F16, tag="f_t", name="f_t")
            acol = pp.tile([128, NT], F32, tag="acol", name="acol")

            with tc.tile_pool(name="xtp", bufs=1) as xtp, \
                 tc.tile_pool(name="vp", bufs=1) as vp, \
                 tc.tile_pool(name="wvp", bufs=1) as wvp, \
                 tc.tile_pool(name="pairw", bufs=3) as pairw, \
                 tc.tile_pool(name="qkpair", bufs=3) as qkpair, \
                 tc.tile_pool(name="epool", bufs=14) as epool, \
                 tc.tile_pool(name="recp", bufs=3) as recp, \
                 tc.tile_pool(name="rbcp", bufs=3) as rbcp:

                xt = [xtp.tile([128, N], F16, tag=f"xt{k}", name=f"xt{k}")
                      for k in range(KT)]
                for k in range(KT):
                    # alternate DMA queues so the xT stream isn't serialized
                    # on one ring
                    (nc.sync if k % 2 == 0 else nc.scalar).dma_start(
                        out=xt[k], in_=xt_d[k * 128:(k + 1) * 128, :])
                vt = [vp.tile([128, VW], F16, tag=f"vt{n}", name=f"vt{n}")
                      for n in range(NT)]
                wv_t = [wvp.tile([128, VW], F16, tag=f"wv{k}", name=f"wv{k}")
                        for k in range(KT)]
                wo_t = [wop.tile([128, C], F16, tag=f"wo{k}", name=f"wo{k}")
                        for k in range(KT)]
                sq = [sqp.tile([128, N], F16, tag=f"sq{k}", name=f"sq{k}")
                      for k in range(KT)]

                with tc.tile_pool(name="ps_s", bufs=2, space="PSUM") as ps_s:
                    # PE warm-up: dummy fp32 matmuls (scratch PSUM, never
                    # read) fill the unavoidable ~6us initial DMA-latency
                    # window so the PE p-state is fully ramped when the
                    # first real matmuls arrive.  Sized to end just before
                    # the first weight DMA completes (~11.5us).
                    with tc.tile_pool(name="ps_warm", bufs=1,
                                      space="PSUM") as ps_warm:
                        warm = pp.tile([1, 128], F32, tag="warm", name="warm")
                        nc.vector.memset(warm, 1.0)
                        wpsum = ps_warm.tile([1, 128], F32, tag="w", name="w")
                        for _ in range(12):
                            nc.tensor.matmul(
                                out=wpsum, lhsT=warm[0:1, 0:1], rhs=warm,
                                start=True, stop=True, skip_group_check=True)

                    qk_tiles = {}
                    proj_emitted = set()

                    def load_pair_w(p):
                        pw = [pairw.tile([128, 256], F16, tag=f"pw{k}",
                                         name=f"pw{k}") for k in range(KT)]
                        for k in range(KT):
                            wdma(
                                out=pw[k],
                                in_=wqk_d[k * 128:(k + 1) * 128,
                                          p * 256:(p + 1) * 256])
                        qt = qkpair.tile([128, N], F16, tag="qtp", name="qtp")
                        kt = qkpair.tile([128, N], F16, tag="ktp", name="ktp")
                        qk_tiles[p] = (pw, qt, kt)

                    def emit_proj_group(pool, p, which, ch):
                        pw, qt, kt = qk_tiles[p]
                        off = 0 if which == "q" else 128
                        pg = pool.tile([128, 512], F32, tag="sp", name="pg")
                        for k in range(KT):
                            nc.tensor.matmul(
                                out=pg, lhsT=pw[k][:, off:off + 128],
                                rhs=xt[k][:, ch * 512:(ch + 1) * 512],
                                start=(k == 0), stop=(k == KT - 1),
                            )
                        if which == "q":
                            nc.vector.tensor_scalar(
                                out=qt[:, ch * 512:(ch + 1) * 512], in0=pg,
                                scalar1=bqs_t[p], scalar2=None, op0=OP.add,
                            )
                        else:
                            nc.vector.tensor_copy(
                                out=kt[:, ch * 512:(ch + 1) * 512], in_=pg)
                        proj_emitted.add((p, which, ch))

                    # scores + exp for one (head, m-tile), emitted one head
                    # ahead of the PV stream; e tiles parked in e_cache
                    e_cache = {}

                    def emit_scores_exp(hid, mt):
                        p, hh = divmod(hid, 2)
                        assert (p, "q", 0) in proj_emitted and \
                            (p, "q", 1) in proj_emitted and \
                            (p, "k", 0) in proj_emitted, (hid, mt)
                        if mt >= 4:
                            assert (p, "k", 1) in proj_emitted, (hid, mt)
                        _, qt, kt = qk_tiles[p]
                        hr = hh * 64
                        sp = ps_s.tile([128, N], F32, tag="sp", name="sp")
                        for ch in range(2):
                            nc.tensor.matmul(
                                out=sp[:, ch * 512:(ch + 1) * 512],
                                lhsT=kt[hr:hr + 64, mt * 128:(mt + 1) * 128],
                                rhs=qt[hr:hr + 64, ch * 512:(ch + 1) * 512],
                                start=True, stop=True,
                            )
                        e = epool.tile([128, N], F16, tag="e", name="e")
                        nc.scalar.activation(out=e, in_=sp, func=AF.Exp)
                        e_cache[(hid, mt)] = e

                    load_pair_w(0)
                    # small consts after the latency-critical xT/pw loads
                    nc.sync.dma_start(out=extra_t, in_=extra_d[:, :])
                    for m in range(NP):
                        nc.sync.dma_start(
                            out=bqs_t[m],
                            in_=bqs_d[m * 128:(m + 1) * 128].unsqueeze(1))
                    nc.sync.dma_start(out=ones_col, in_=ones_d[:].unsqueeze(1))
                    nc.sync.dma_start(
                        out=bve_bc,
                        in_=bass.AP(tensor=bve_d[:].tensor, offset=0,
                                    ap=[[0, 128], [1, VW]]))
                    nc.sync.dma_start(
                        out=bo_bc,
                        in_=bass.AP(tensor=extra_d[:, :].tensor, offset=C,
                                    ap=[[0, 128], [1, C]]))
                    for k in range(KT):
                        wdma(out=wv_t[k], in_=wv_d[k * 128:(k + 1) * 128, :])
                    for k in range(KT):
                        wdma(out=wo_t[k], in_=wo_d[k * 128:(k + 1) * 128, :])

                    # ---- prologue: pair-0 q/k projections + v projection
                    # interleaved with head-0 scores/exp.  Scores are emitted
                    # as early as their q/k evictions allow so the ScalarE
                    # exp stream starts ~10us in; v-chunks and projection
                    # groups go on their own prologue-scoped PSUM ring. ----
                    with tc.tile_pool(name="ps_vp", bufs=3,
                                      space="PSUM") as ps_vp:

                        def emit_vchunk(n, c0, cw):
                            pv = ps_vp.tile([128, 512], F32, tag="sp", name="pv")
                            for k in range(KT):
                                nc.tensor.matmul(
                                    out=pv[:, 0:cw],
                                    lhsT=xt[k][:, n * 128:(n + 1) * 128],
                                    rhs=wv_t[k][:, c0:c0 + cw],
                                    start=(k == 0), stop=(k == KT - 1),
                                )
                            nc.vector.tensor_tensor(
                                out=vt[n][:, c0:c0 + cw], in0=pv[:, 0:cw],
                                in1=bve_bc[:, c0:c0 + cw], op=OP.add)

                        for which, ch in (("q", 0), ("q", 1), ("k", 0)):
                            emit_proj_group(ps_vp, 0, which, ch)
                        emit_scores_exp(0, 0)
                        emit_scores_exp(0, 1)
                        emit_proj_group(ps_vp, 0, "k", 1)
                        emit_scores_exp(0, 2)
                        emit_scores_exp(0, 3)
                        vchunks = [(n, c0, cw) for n in range(NT)
                                   for c0, cw in ((0, 512), (512, VW - 512))]
                        for i, (n, c0, cw) in enumerate(vchunks):
                            emit_vchunk(n, c0, cw)
                            if i in (1, 3, 5, 7):
                                emit_scores_exp(0, 4 + i // 2)

                    def epilogue(hid, pa):
                        p, hh = divmod(hid, 2)
                        hr = hh * 64
                        # denominator row -> SBUF copy (the custom-DVE approx
                        # recip NaNs on hw when reading PSUM) -> fast approx
                        # reciprocal -> DRAM-bounced broadcast -> divide
                        # straight out of PSUM.  Denoms are sums of 1024
                        # positive exps (>= ~50): approx-recip edge cases
                        # can't trigger.
                        den = recp.tile([1, N], F32, tag="den", name="den")
                        if hid == H - 1:
                            # exp stream has dried up: use the idle ScalarE
                            # so the DVE tail doesn't back up
                            nc.scalar.copy(out=den, in_=pa[64:65, :])
                        else:
                            nc.vector.tensor_copy(out=den, in_=pa[64:65, :])
                        rec = recp.tile([1, N], F32, tag="rec", name="rec")
                        nc.vector.reciprocal_approx_fast(out=rec, in_=den)
                        nc.sync.dma_start(out=rscr[hid:hid + 1, :], in_=rec)
                        rbc = rbcp.tile([64, N], F32, tag="rbc", name="rbc")
                        src = rscr[hid:hid + 1, :]
                        nc.sync.dma_start(
                            out=rbc,
                            in_=bass.AP(tensor=src.tensor, offset=src.offset,
                                        ap=[[0, 64]] + [list(d) for d in src.ap[1:]]),
                        )
                        nc.vector.tensor_tensor(
                            out=att[p][hr:hr + 64, :], in0=pa[0:64, :],
                            in1=rbc, op=OP.mult)
                        if hh == 1:
                            if p == NP - 1:
                                nc.scalar.activation(out=sq[p], in_=att[p],
                                                     func=AF.Square)
                            else:
                                nc.vector.tensor_tensor(out=sq[p], in0=att[p],
                                                        in1=att[p], op=OP.mult)

                    # ---- main pipeline: PV(H) with scores/exp(H+1) and pair
                    # p+1's projection groups slotted into (p, h0) ----
                    with tc.tile_pool(name="ps_att", bufs=2,
                                      space="PSUM") as ps_att:
                        for hid in range(H):
                            p, hh = divmod(hid, 2)
                            if hh == 0 and p + 1 < NP:
                                load_pair_w(p + 1)
                            pa = ps_att.tile([65, N], F32, tag="pa", name="pa")
                            for mt in range(NT):
                                e = e_cache.pop((hid, mt))
                                for ch in range(2):
                                    nc.tensor.matmul(
                                        out=pa[:, ch * 512:(ch + 1) * 512],
                                        lhsT=vt[mt][:, hid * 65:(hid + 1) * 65],
                                        rhs=e[:, ch * 512:(ch + 1) * 512],
                                        start=(mt == 0), stop=(mt == NT - 1),
                                    )
                                if hh == 0 and p + 1 < NP and mt in (1, 3, 5, 7):
                                    emit_proj_group(
                                        ps_s, p + 1,
                                        "q" if mt in (1, 3) else "k",
                                        0 if mt in (1, 5) else 1)
                                if hid + 1 < H:
                                    emit_scores_exp(hid + 1, mt)
                            epilogue(hid, pa)

            # ---- phase C: LN stats + output projection ----
            with tc.tile_pool(name="rowpool", bufs=1) as rowpool, \
                 tc.tile_pool(name="ypool", bufs=3) as ypool:

                with tc.tile_pool(name="ps_row", bufs=1, space="PSUM") as ps_row:
                    rows = {}
                    for nm in ("sx0", "sx1", "sxx0", "sxx1"):
                        rows[nm] = ps_row.tile([1, 512], F32, tag=nm, name=nm)
                    for ch in range(2):
                        for k in range(KT):
                            nc.tensor.matmul(
                                out=rows[f"sx{ch}"], lhsT=ones_col,
                                rhs=att[k][:, ch * 512:(ch + 1) * 512],
                                start=(k == 0), stop=(k == KT - 1),
                            )
                        for k in range(KT):
                            nc.tensor.matmul(
                                out=rows[f"sxx{ch}"], lhsT=ones_col,
                                rhs=sq[k][:, ch * 512:(ch + 1) * 512],
                                start=(k == 0), stop=(k == KT - 1),
                            )
                    mrow = rowpool.tile([1, N], F32, tag="mrow", name="mrow")
                    t0 = rowpool.tile([1, N], F32, tag="t0", name="t0")
                    for ch in range(2):
                        sl = slice(ch * 512, (ch + 1) * 512)
                        nc.scalar.mul(out=mrow[:, sl], in_=rows[f"sx{ch}"], mul=1.0 / C)
                        nc.scalar.mul(out=f_t[0:1, sl], in_=rows[f"sx{ch}"], mul=-1.0 / C)
                        # t0 = sxx/C + eps so varr below is exactly var+eps
                        nc.scalar.activation(out=t0[:, sl], in_=rows[f"sxx{ch}"],
                                             func=AF.Identity, bias=eps_row,
                                             scale=1.0 / C)
                    m2 = rowpool.tile([1, N], F32, tag="m2", name="m2")
                    nc.vector.tensor_tensor(out=m2, in0=mrow, in1=mrow, op=OP.mult)
                    varr = rowpool.tile([1, N], F32, tag="varr", name="varr")
                    nc.vector.tensor_tensor(out=varr, in0=t0, in1=m2, op=OP.subtract)
                    # per-token rstd in [128, NT] shape: DMA-bounce transpose
                    # of var+eps, then sqrt + fast approx reciprocal with all
                    # 128 lanes active (token t = j*128 + p -> [p, j])
                    nc.sync.dma_start(out=ascr[:].unsqueeze(0), in_=varr)
                    vcol = rowpool.tile([128, NT], F32, tag="vcol", name="vcol")
                    nc.sync.dma_start(
                        out=vcol,
                        in_=bass.AP(tensor=ascr[:].tensor, offset=0,
                                    ap=[[1, 128], [128, NT]]),
                    )
                    scol = rowpool.tile([128, NT], F32, tag="scol", name="scol")
                    nc.scalar.activation(out=scol, in_=vcol, func=AF.Sqrt)
                    nc.vector.reciprocal_approx_fast(out=acol, in_=scol)

                with tc.tile_pool(name="ps_out", bufs=4, space="PSUM") as ps_out:
                    for n in range(NT):
                        po = ps_out.tile([128, C], F32, tag="po", name="po")
                        for c0, cw in ((0, 512), (512, C - 512)):
                            for k in range(KT):
                                nc.tensor.matmul(
                                    out=po[:, c0:c0 + cw],
                                    lhsT=att[k][:, n * 128:(n + 1) * 128],
                                    rhs=wo_t[k][:, c0:c0 + cw],
                                    start=(k == 0), stop=False,
                                )
                            nc.tensor.matmul(
                                out=po[:, c0:c0 + cw],
                                lhsT=f_t[:, n * 128:(n + 1) * 128],
                                rhs=extra_t[0:1, c0:c0 + cw],
                                start=False, stop=True,
                            )
                        yt = ypool.tile([128, C], F32, tag="yt", name="yt")
                        # yt = (po * rstd[token]) + bo_eff
                        nc.vector.scalar_tensor_tensor(
                            out=yt, in0=po, scalar=acol[:, n:n + 1],
                            in1=bo_bc, op0=OP.mult, op1=OP.add)
                        nc.sync.dma_start(out=y_d[n * 128:(n + 1) * 128, :], in_=yt)

    nc.compile()
    return nc


def prepare_in_maps(x, Wq, bq, Wk, bk, Wv, bv, Wo, bo, ln_g, ln_b):
    x = np.asarray(x, np.float32)
    Wq = np.asarray(Wq, np.float32); bq = np.asarray(bq, np.float32)
    Wk = np.asarray(Wk, np.float32)
    Wv = np.asarray(Wv, np.float32); bv = np.asarray(bv, np.float32)
    Wo = np.asarray(Wo, np.float32); bo = np.asarray(bo, np.float32)
    ln_g = np.asarray(ln_g, np.float32); ln_b = np.asarray(ln_b, np.float32)

    wq = np.ascontiguousarray(Wq.T) * SCALE
    wk = np.ascontiguousarray(Wk.T)
    wv = np.ascontiguousarray(Wv.T)            # [C, C]
    wv_ext = np.zeros((C, VW), np.float32)
    bve = np.zeros((VW,), np.float32)
    for h in range(H):
        wv_ext[:, h * 65: h * 65 + 64] = wv[:, h * 64:(h + 1) * 64]
        bve[h * 65: h * 65 + 64] = bv[h * 64:(h + 1) * 64]
        bve[h * 65 + 64] = 1.0                 # ones column for denominators
    # pair-blocked q/k weights: [wq_p | wk_p] per 128-channel head pair
    wqk = np.zeros((C, 2 * C), np.float32)
    for p in range(NP):
        wqk[:, p * 256: p * 256 + 128] = wq[:, p * 128:(p + 1) * 128]
        wqk[:, p * 256 + 128:(p + 1) * 256] = wk[:, p * 128:(p + 1) * 128]
    wo = ln_g[:, None] * np.ascontiguousarray(Wo.T)
    bo_eff = bo + ln_b @ Wo.T
    extra = np.stack([wo.sum(axis=0), bo_eff]).astype(np.float32)
    bqs = bq * SCALE

    f16 = np.float16
    shared = {"wqk": wqk.astype(f16), "wv": wv_ext.astype(f16),
              "wo": wo.astype(f16), "extra": extra.astype(f16),
              "bqs": bqs, "bve": bve.astype(f16),
              "onesd": np.ones(128, f16)}
    in_maps = []
    for b in range(B):
        xT = np.ascontiguousarray(x[b].T).astype(f16)   # [C, N]
        in_maps.append({"xt": xT, **shared})
    return in_maps


_NC_CACHE = []


def _get_nc():
    if not _NC_CACHE:
        _NC_CACHE.append(build_kernel())
    return _NC_CACHE[0]


def kernel(**inputs) -> np.ndarray:
    nc = _get_nc()
    in_maps = prepare_in_maps(**inputs)
    res = run_bass_kernel_spmd(nc, in_maps, core_ids=list(range(B)))
    return np.stack([res.results[b]["y"] for b in range(B)], axis=0)
